# revision 2
# baseline (speedup 1.0000x reference)
"""Group-limited MoE router kernel for Trainium2 (Bass/Tile), 8-core SPMD.

Implements, per token (row of 256 experts):
  scores = sigmoid(logits); biased = scores + bias
  group_score[g] = top2sum(biased[g*32:(g+1)*32]) for 8 groups
  keep top-4 groups, mask the rest to -inf
  topk_ids = top-8 of masked biased (descending)
  weights  = scores[topk_ids]; renormalize to sum 1; * 2.5

Data-parallel over tokens: 131072 tokens -> 8 cores x 16384.
Layout: tokens on SBUF partitions (128/slab), experts on the free dim.
"""

import numpy as np

TOKENS = 131072
E = 256
G = 8
EPG = 32
K = 8
KG = 4
SCALE = 2.5
N_CORES = 8
TPC = TOKENS // N_CORES

NEG = -1.0e30  # group mask value


def build_kernel(tpc: int):
    import concourse.bass as bass
    import concourse.bacc as bacc
    import concourse.mybir as mybir
    from concourse.tile import TileContext

    f32 = mybir.dt.float32
    u32 = mybir.dt.uint32

    nc = bacc.Bacc()
    logits_d = nc.declare_dram_parameter("logits", [tpc, E], f32, isOutput=False)
    bias_d = nc.declare_dram_parameter("bias", [1, E], f32, isOutput=False)
    w_d = nc.declare_dram_parameter("weights", [tpc, K], f32, isOutput=True)
    i_d = nc.declare_dram_parameter("ids", [tpc, K], u32, isOutput=True)

    P = 128
    n_slab = tpc // P
    Sigmoid = mybir.ActivationFunctionType.Sigmoid
    Alu = mybir.AluOpType

    with TileContext(nc) as tc:
        with (
            tc.tile_pool(name="const", bufs=1) as const_pool,
            tc.tile_pool(name="big", bufs=3) as big,
            tc.tile_pool(name="small", bufs=4) as small,
            tc.tile_pool(name="out", bufs=4) as outp,
        ):
            bias_sb = const_pool.tile([P, E], f32)
            nc.gpsimd.dma_start(out=bias_sb, in_=bias_d[:].to_broadcast([P, E]))
            # pre-touch on DVE so later consumers carry at most one sync wait
            dummy = const_pool.tile([P, 1], f32)
            nc.vector.tensor_copy(out=dummy, in_=bias_sb[:, 0:1])

            for s in range(n_slab):
                t0 = s * P
                x = big.tile([P, E], f32, tag="x")
                nc.sync.dma_start(out=x, in_=logits_d[t0 : t0 + P, :])

                # match jax-on-neuron sigmoid bit-exactly: 1/(1+exp(-x))
                ex = big.tile([P, E], f32, tag="ex")
                nc.scalar.activation(
                    out=ex, in_=x, func=mybir.ActivationFunctionType.Exp, scale=-1.0
                )
                nc.scalar.add(out=ex, in_=ex, add=1.0)
                scores = big.tile([P, E], f32, tag="scores")
                nc.vector.reciprocal(out=scores, in_=ex)

                biased = big.tile([P, E], f32, tag="biased")
                nc.vector.tensor_tensor(
                    out=biased, in0=scores, in1=bias_sb, op=Alu.add
                )

                # --- group scores: top1 + top2 per group of 32 ---
                bg = biased.rearrange("p (g e) -> p g e", g=G)
                m1 = small.tile([P, G], f32, tag="m1")
                nc.vector.tensor_reduce(
                    out=m1, in_=bg, axis=mybir.AxisListType.X, op=Alu.max
                )
                rep = big.tile([P, E], f32, tag="rep")
                nc.vector.match_replace(
                    out=rep, in_to_replace=m1, in_values=biased, imm_value=NEG
                )
                m2 = small.tile([P, G], f32, tag="m2")
                nc.vector.tensor_reduce(
                    out=m2,
                    in_=rep.rearrange("p (g e) -> p g e", g=G),
                    axis=mybir.AxisListType.X,
                    op=Alu.max,
                )
                gs = small.tile([P, G], f32, tag="gs")
                nc.vector.tensor_tensor(out=gs, in0=m1, in1=m2, op=Alu.add)

                # --- select top-4 groups: threshold at 4th largest ---
                g8 = small.tile([P, 8], f32, tag="g8")
                nc.vector.max(out=g8, in_=gs)
                # neg[g] = (gs[g] < t) * NEG   (0 for kept groups)
                neg = small.tile([P, G], f32, tag="neg")
                nc.vector.tensor_scalar(
                    out=neg,
                    in0=gs,
                    scalar1=g8[:, 3:4],
                    scalar2=NEG,
                    op0=Alu.is_lt,
                    op1=Alu.mult,
                )
                masked = big.tile([P, E], f32, tag="masked")
                nc.vector.tensor_tensor(
                    out=masked,
                    in0=biased,
                    in1=neg.unsqueeze(2).to_broadcast([P, G, EPG]),
                    op=Alu.add,
                )

                # --- top-8 of masked biased: values + expert ids ---
                vals8 = small.tile([P, K], f32, tag="vals8")
                nc.vector.max(out=vals8, in_=masked)
                idx8 = small.tile([P, K], u32, tag="idx8")
                nc.vector.max_index(out=idx8, in_max=vals8, in_values=masked)

                # --- gather scores at the top-8 positions ---
                # indicator of the 8 winning positions
                ind = big.tile([P, E], f32, tag="ind")
                nc.vector.tensor_scalar(
                    out=ind,
                    in0=masked,
                    scalar1=vals8[:, 7:8],
                    scalar2=None,
                    op0=Alu.is_ge,
                )
                sel = big.tile([P, E], f32, tag="sel")
                nc.vector.tensor_tensor(out=sel, in0=scores, in1=ind, op=Alu.mult)
                s8 = small.tile([P, K], f32, tag="s8")
                nc.vector.max(out=s8, in_=sel)
                sidx8 = small.tile([P, K], u32, tag="sidx8")
                nc.vector.max_index(out=sidx8, in_max=s8, in_values=sel)

                # --- associate score-sorted (s8, sidx8) to rank order idx8 ---
                # C[p,k,j] = (idx8[p,k] == sidx8[p,j]); w8[p,k] = sum_j C*s8[p,j]
                idx8f = small.tile([P, K], f32, tag="idx8f")
                nc.scalar.copy(out=idx8f, in_=idx8)
                sidx8f = small.tile([P, K], f32, tag="sidx8f")
                nc.scalar.copy(out=sidx8f, in_=sidx8)
                cmat = small.tile([P, K, K], f32, tag="cmat")
                nc.vector.tensor_tensor(
                    out=cmat,
                    in0=idx8f.unsqueeze(2).to_broadcast([P, K, K]),
                    in1=sidx8f.unsqueeze(1).to_broadcast([P, K, K]),
                    op=Alu.is_equal,
                )
                w64 = small.tile([P, K, K], f32, tag="w64")
                nc.vector.tensor_tensor(
                    out=w64,
                    in0=cmat,
                    in1=s8.unsqueeze(1).to_broadcast([P, K, K]),
                    op=Alu.mult,
                )
                w8 = outp.tile([P, K], f32, tag="w8")
                nc.vector.tensor_reduce(
                    out=w8, in_=w64, axis=mybir.AxisListType.X, op=Alu.add
                )

                # --- renormalize: w * SCALE / (sum + 1e-20) ---
                wsum = small.tile([P, 1], f32, tag="wsum")
                nc.vector.tensor_reduce(
                    out=wsum, in_=w8, axis=mybir.AxisListType.X, op=Alu.add
                )
                nc.vector.tensor_scalar(
                    out=wsum,
                    in0=wsum,
                    scalar1=1.0e-20,
                    scalar2=None,
                    op0=Alu.add,
                )
                rcp = small.tile([P, 1], f32, tag="rcp")
                nc.vector.reciprocal(out=rcp, in_=wsum)
                nc.vector.tensor_scalar(
                    out=rcp,
                    in0=rcp,
                    scalar1=SCALE,
                    scalar2=None,
                    op0=Alu.mult,
                )
                wout = outp.tile([P, K], f32, tag="wout")
                nc.vector.tensor_scalar(
                    out=wout,
                    in0=w8,
                    scalar1=rcp,
                    scalar2=None,
                    op0=Alu.mult,
                )

                ids_out = outp.tile([P, K], u32, tag="ids_out")
                nc.vector.tensor_copy(out=ids_out, in_=idx8)

                nc.sync.dma_start(out=w_d[t0 : t0 + P, :], in_=wout)
                nc.sync.dma_start(out=i_d[t0 : t0 + P, :], in_=ids_out)

    nc.finalize()
    return nc


_NC_CACHE = {}


def _get_nc(tpc: int):
    if tpc not in _NC_CACHE:
        _NC_CACHE[tpc] = build_kernel(tpc)
    return _NC_CACHE[tpc]


def kernel(router_logits: np.ndarray, expert_bias: np.ndarray, _trace: bool = False):
    from concourse.bass_utils import run_bass_kernel_spmd

    router_logits = np.asarray(router_logits, dtype=np.float32)
    expert_bias = np.asarray(expert_bias, dtype=np.float32)
    tokens = router_logits.shape[0]
    assert tokens % N_CORES == 0
    tpc = tokens // N_CORES

    nc = _get_nc(tpc)
    bias_in = expert_bias.reshape(1, E)
    in_maps = [
        {
            "logits": np.ascontiguousarray(
                router_logits[c * tpc : (c + 1) * tpc]
            ),
            "bias": bias_in,
        }
        for c in range(N_CORES)
    ]
    res = run_bass_kernel_spmd(
        nc, in_maps, core_ids=list(range(N_CORES)), trace=_trace
    )
    weights = np.concatenate([r["weights"] for r in res.results], axis=0)
    ids = np.concatenate([r["ids"] for r in res.results], axis=0).astype(np.int32)
    if _trace:
        kernel.last_exec_time_ns = res.exec_time_ns
        kernel.last_mean_exec_time_ns = res.mean_exec_time_ns
        it = res.instructions_and_trace
        kernel.last_trace_path = it[1] if it else None
    return weights, ids

